# revision 11
# baseline (speedup 1.0000x reference)
"""Multi-head causal attention ensemble on 8 TRN2 NeuronCores.

Problem: x [2, 2048, 1024], 16 heads of dim 64, per-head QKV projections,
causal softmax attention, concat, output projection [1024 -> 1024] + bias.

Sharding: 2-way data parallel over batch x 4-way tensor parallel over heads.
Core c handles batch c//4 and heads [4*(c%4), 4*(c%4)+4). Each core computes
its partial output-projection contribution; the host sums the 4 partials per
batch and adds the bias (the all-reduce equivalent of TP unsharding).

Device kernel per core (S=2048, D=1024, 4 heads = 2 head-pairs):
  - qT/kT computed transposed [head-pair 128, S] so scores come out as
    sT[k, q] = kT^T @ qT directly (no transposes anywhere).
  - softmax without max-subtraction (scores are N(0,1)-ish; exp is safe in
    fp32), exp on ScalarE with the 1/sqrt(64) scale fused into the
    activation's free affine.
  - causal masking: lower-triangular staircase masks multiply the exp'd
    probabilities of diagonal blocks (exact zeros), off-diagonal blocks
    above the diagonal are simply never computed.
  - AV matmul uses lhsT = [v_h | ones] so the softmax denominator lands in
    row 64 of the PSUM accumulator for free.
  - normalization: DVE reciprocal of the denominator row, broadcast across
    64 partitions via a K=1 outer-product matmul, DVE multiply.
  - output projection consumes the transposed AV output directly as lhsT.
"""

import sys

sys.path.insert(0, "/opt/trn_rl_repo")
import numpy as np

import concourse.bass as bass  # noqa: F401  (registers AP types)
import concourse.tile as tile
from concourse import bacc, mybir
from concourse.bass_utils import run_bass_kernel_spmd

B, S, D, H, HD = 2, 2048, 1024, 16, 64
NCORES = 8
HPC = 4  # heads per core
NPAIR = 2  # head-pairs per core
QC = 4  # 512-wide q chunks
F32 = mybir.dt.float32

# compute dtype mode: "f32" | "f32r" | "bf16"
MODE = "bf16"

_BUILD_CACHE = {}


def _np_cdt(mode):
    if mode == "bf16":
        import ml_dtypes

        return np.dtype(ml_dtypes.bfloat16)
    return np.dtype(np.float32)


def _build(mode, iters=1):
    """Build + compile the single-core Bass program (SPMD across 8 cores).

    iters > 1 wraps the body in a device-side loop (benchmark amplification).
    """
    cdt = mybir.dt.bfloat16 if mode == "bf16" else F32
    nc = bacc.Bacc("TRN2", target_bir_lowering=False, debug=False)

    xT_d = nc.dram_tensor("xT", [D, S], cdt, kind="ExternalInput")
    wq_d = nc.dram_tensor("wq", [D, HPC * HD], cdt, kind="ExternalInput")
    wk_d = nc.dram_tensor("wk", [D, HPC * HD], cdt, kind="ExternalInput")
    wv_d = nc.dram_tensor("wv", [D, HPC * HD], cdt, kind="ExternalInput")
    wo_d = nc.dram_tensor("wo", [HPC * HD, D], cdt, kind="ExternalInput")
    mask_d = nc.dram_tensor("mask0", [128, 512], cdt, kind="ExternalInput")
    out_d = nc.dram_tensor("out", [S, D], F32, kind="ExternalOutput")

    def mm(out, lhsT, rhs, **kw):
        if mode == "f32r":
            lhsT = lhsT.bitcast(mybir.dt.float32r)
            rhs = rhs.bitcast(mybir.dt.float32r)
        return nc.tensor.matmul(out, lhsT, rhs, **kw)

    EXP = mybir.ActivationFunctionType.Exp

    with tile.TileContext(nc) as tc:
        with (
            tc.tile_pool(name="pin", bufs=1) as pin,
            tc.tile_pool(name="work", bufs=4) as work,
            tc.tile_pool(name="psA", bufs=2, space="PSUM") as psA,
            tc.tile_pool(name="psS", bufs=4, space="PSUM") as psS,
            tc.tile_pool(name="psV", bufs=2, space="PSUM") as psV,
        ):
            if iters > 1:
                loop_cm = tc.For_i(0, iters, 1)
                loop_cm.__enter__()
            # ---- persistent inputs
            xT = pin.tile([128, 8, S], cdt, tag="xT")
            nc.sync.dma_start(xT[:], xT_d.ap().rearrange("(t p) s -> p t s", p=128))
            wq = pin.tile([128, 8, 256], cdt, tag="wq")
            nc.sync.dma_start(wq[:], wq_d.ap().rearrange("(t p) e -> p t e", p=128))
            wk = pin.tile([128, 8, 256], cdt, tag="wk")
            nc.sync.dma_start(wk[:], wk_d.ap().rearrange("(t p) e -> p t e", p=128))
            wv = pin.tile([128, 8, 256], cdt, tag="wv")
            nc.sync.dma_start(wv[:], wv_d.ap().rearrange("(t p) e -> p t e", p=128))
            wo = pin.tile([128, 2, D], cdt, tag="wo")
            nc.sync.dma_start(wo[:], wo_d.ap().rearrange("(t p) d -> p t d", p=128))
            mask0 = pin.tile([128, 512], cdt, tag="mask0")
            nc.sync.dma_start(mask0[:], mask_d.ap())

            ones64 = pin.tile([1, 64], cdt, tag="ones64")
            nc.vector.memset(ones64[:], 1.0)

            # persistent intermediates
            qT = pin.tile([128, NPAIR, S], cdt, tag="qT")
            kT = pin.tile([128, NPAIR, S], cdt, tag="kT")
            vsb = pin.tile([128, 16, HPC, 65], cdt, tag="vsb")
            outcat = pin.tile([128, NPAIR, S], cdt, tag="outcat")

            # prime the exp activation table load during the projection phase
            prime = work.tile([1, 1], F32, tag="prime")
            nc.scalar.activation(prime[:], ones64[0:1, 0:1], EXP)

            # ---- q/k projections (transposed layout, head-pairs packed)
            for p in range(NPAIR):
                for w, dst in ((wq, qT), (wk, kT)):
                    for nch in range(4):
                        ps = psA.tile([128, 512], F32, tag="psA")
                        for kt in range(8):
                            mm(
                                ps[:],
                                w[:, kt, p * 128 : (p + 1) * 128],
                                xT[:, kt, nch * 512 : (nch + 1) * 512],
                                start=(kt == 0),
                                stop=(kt == 7),
                            )
                        nc.scalar.copy(dst[:, p, nch * 512 : (nch + 1) * 512], ps[:])

            # ---- v projection (natural layout, all 4 heads, ones col appended)
            for st in range(16):
                ps = psA.tile([128, 256], F32, tag="psA")
                for kt in range(8):
                    mm(
                        ps[:],
                        xT[:, kt, st * 128 : (st + 1) * 128],
                        wv[:, kt, :],
                        start=(kt == 0),
                        stop=(kt == 7),
                    )
                nc.vector.tensor_copy(
                    vsb[:, st, :, 0:64], ps[:].rearrange("p (h e) -> p h e", h=HPC)
                )
                nc.vector.memset(vsb[:, st, :, 64:65], 1.0)

            # ---- attention
            for p in range(NPAIR):
                for qc in range(QC):
                    kmax = 4 * qc + 3
                    avs = [
                        psV.tile([65, 512], F32, tag="av", name=f"av{p}_{qc}_{h}")
                        for h in range(2)
                    ]
                    for kb in range(kmax + 1):
                        m = kb - 4 * qc
                        off = 128 * m if m > 0 else 0
                        n = 512 - off
                        for h in range(2):
                            sps = psS.tile([128, 512], F32, tag="sps")
                            mm(
                                sps[:, 0:n],
                                kT[h * 64 : (h + 1) * 64, p, kb * 128 : (kb + 1) * 128],
                                qT[
                                    h * 64 : (h + 1) * 64,
                                    p,
                                    qc * 512 + off : (qc + 1) * 512,
                                ],
                            )
                            pt = work.tile([128, 512], cdt, tag="pt")
                            nc.scalar.activation(pt[:, 0:n], sps[:, 0:n], EXP, scale=0.125)
                            if m >= 0:
                                nc.vector.tensor_mul(
                                    pt[:, 0:n], pt[:, 0:n], mask0[:, 0:n]
                                )
                            mm(
                                avs[h][:, off:512],
                                vsb[:, kb, p * 2 + h, :],
                                pt[:, 0:n],
                                start=(kb == 0),
                                stop=(kb == kmax),
                            )
                    # normalize: outcat[h] = av[0:64] / av[64]
                    for h in range(2):
                        rec = work.tile([1, 512], F32, tag="rec")
                        nc.vector.reciprocal(rec[:], avs[h][64:65, :])
                        recc = work.tile([1, 512], cdt, tag="recc")
                        nc.vector.tensor_copy(recc[:], rec[:])
                        bc_ps = psS.tile([64, 512], F32, tag="sps")
                        mm(bc_ps[:], ones64[:], recc[:])
                        bc_sb = work.tile([64, 512], F32, tag="bcsb")
                        nc.scalar.copy(bc_sb[:], bc_ps[:])
                        nc.vector.tensor_mul(
                            outcat[h * 64 : (h + 1) * 64, p, qc * 512 : (qc + 1) * 512],
                            avs[h][0:64, :],
                            bc_sb[:],
                        )

            # ---- output projection
            for st in range(16):
                for dc in range(2):
                    ps = psA.tile([128, 512], F32, tag="psA")
                    for p in range(NPAIR):
                        mm(
                            ps[:],
                            outcat[:, p, st * 128 : (st + 1) * 128],
                            wo[:, p, dc * 512 : (dc + 1) * 512],
                            start=(p == 0),
                            stop=(p == NPAIR - 1),
                        )
                    ysb = work.tile([128, 512], F32, tag="ysb")
                    nc.vector.tensor_copy(ysb[:], ps[:])
                    nc.sync.dma_start(
                        out_d.ap()[
                            st * 128 : (st + 1) * 128, dc * 512 : (dc + 1) * 512
                        ],
                        ysb[:],
                    )

            if iters > 1:
                loop_cm.__exit__(None, None, None)

    nc.compile()
    return nc


def _masks_np(dtype):
    """mask0[kl, f] = 1.0 where f >= kl else 0 (block-causal staircase)."""
    kl = np.arange(128)[:, None]
    f = np.arange(512)[None, :]
    return (f >= kl).astype(dtype)


def kernel(x, Wq, Wk, Wv, Wo, bo):
    mode = MODE
    if mode not in _BUILD_CACHE:
        _BUILD_CACHE[mode] = _build(mode)
    nc = _BUILD_CACHE[mode]
    cdt = _np_cdt(mode)

    x = np.asarray(x, dtype=np.float32)
    Wq = np.asarray(Wq, dtype=np.float32)
    Wk = np.asarray(Wk, dtype=np.float32)
    Wv = np.asarray(Wv, dtype=np.float32)
    Wo = np.asarray(Wo, dtype=np.float32)
    bo = np.asarray(bo, dtype=np.float32)

    masks = _masks_np(cdt)
    in_maps = []
    for c in range(NCORES):
        b, j = divmod(c, 4)
        hs = slice(4 * j, 4 * j + 4)
        in_maps.append(
            {
                "xT": np.ascontiguousarray(x[b].T).astype(cdt),
                "wq": Wq[hs].transpose(1, 0, 2).reshape(D, HPC * HD).astype(cdt),
                "wk": Wk[hs].transpose(1, 0, 2).reshape(D, HPC * HD).astype(cdt),
                "wv": Wv[hs].transpose(1, 0, 2).reshape(D, HPC * HD).astype(cdt),
                "wo": np.ascontiguousarray(Wo[256 * j : 256 * (j + 1)]).astype(cdt),
                "mask0": masks,
            }
        )

    res = run_bass_kernel_spmd(nc, in_maps, core_ids=list(range(NCORES)))
    y = np.zeros((B, S, D), dtype=np.float32)
    for c in range(NCORES):
        y[c // 4] += res.results[c]["out"]
    y += bo
    return y


# revision 17
# speedup vs baseline: 1.4647x; 1.4647x over previous
"""Multi-head causal attention ensemble on 8 TRN2 NeuronCores.

Problem: x [2, 2048, 1024], 16 heads of dim 64, per-head QKV projections,
causal softmax attention, concat, output projection [1024 -> 1024] + bias.

Sharding: 2-way data parallel over batch x 4-way tensor parallel over heads.
Core c handles batch c//4 and heads [4*(c%4), 4*(c%4)+4). Each core computes
its partial output-projection contribution; the host sums the 4 partials per
batch and adds the bias (the all-reduce equivalent of TP unsharding).

Device kernel per core (S=2048, D=1024, 4 heads = 2 head-pairs):
  - qT/kT computed transposed [head-pair 128, S] so scores come out as
    sT[k, q] = kT^T @ qT directly (no transposes anywhere).
  - softmax without max-subtraction (scores are N(0,1)-ish; exp is safe in
    fp32), exp on ScalarE with the 1/sqrt(64) scale fused into the
    activation's free affine.
  - causal masking: lower-triangular staircase masks multiply the exp'd
    probabilities of diagonal blocks (exact zeros), off-diagonal blocks
    above the diagonal are simply never computed.
  - AV matmul uses lhsT = [v_h | ones] so the softmax denominator lands in
    row 64 of the PSUM accumulator for free.
  - normalization: DVE reciprocal of the denominator row, broadcast across
    64 partitions via a K=1 outer-product matmul, DVE multiply.
  - output projection consumes the transposed AV output directly as lhsT.
"""

import sys

sys.path.insert(0, "/opt/trn_rl_repo")
import numpy as np

import concourse.bass as bass  # noqa: F401  (registers AP types)
import concourse.tile as tile
from concourse import bacc, mybir
from concourse.bass_utils import run_bass_kernel_spmd

B, S, D, H, HD = 2, 2048, 1024, 16, 64
NCORES = 8
HPC = 4  # heads per core
NPAIR = 2  # head-pairs per core
QC = 4  # 512-wide q chunks
F32 = mybir.dt.float32

# compute dtype mode: "f32" | "f32r" | "bf16"
MODE = "bf16"

_BUILD_CACHE = {}


def _np_cdt(mode):
    if mode == "bf16":
        import ml_dtypes

        return np.dtype(ml_dtypes.bfloat16)
    return np.dtype(np.float32)


def _build(mode, iters=1):
    """Build + compile the single-core Bass program (SPMD across 8 cores).

    iters > 1 wraps the body in a device-side loop (benchmark amplification).
    """
    cdt = mybir.dt.bfloat16 if mode == "bf16" else F32
    nc = bacc.Bacc("TRN2", target_bir_lowering=False, debug=False)

    xT_d = nc.dram_tensor("xT", [D, S], cdt, kind="ExternalInput")
    wq_d = nc.dram_tensor("wq", [D, HPC * HD], cdt, kind="ExternalInput")
    wk_d = nc.dram_tensor("wk", [D, HPC * HD], cdt, kind="ExternalInput")
    wv_d = nc.dram_tensor("wv", [D, HPC * HD], cdt, kind="ExternalInput")
    wo_d = nc.dram_tensor("wo", [HPC * HD, D], cdt, kind="ExternalInput")
    mask_d = nc.dram_tensor("mask0", [128, 2, 512], cdt, kind="ExternalInput")
    out_d = nc.dram_tensor("out", [S, D], F32, kind="ExternalOutput")

    def mm(out, lhsT, rhs, **kw):
        if mode == "f32r":
            lhsT = lhsT.bitcast(mybir.dt.float32r)
            rhs = rhs.bitcast(mybir.dt.float32r)
        return nc.tensor.matmul(out, lhsT, rhs, **kw)

    EXP = mybir.ActivationFunctionType.Exp

    with tile.TileContext(nc) as tc:
        with (
            tc.tile_pool(name="pin", bufs=1) as pin,
            tc.tile_pool(name="work", bufs=4) as work,
            tc.tile_pool(name="psS", bufs=2, space="PSUM") as psS,
            tc.tile_pool(name="psV", bufs=4, space="PSUM") as psV,
        ):
            if iters > 1:
                loop_cm = tc.For_i(0, iters, 1)
                loop_cm.__enter__()
            # ---- persistent inputs
            xT = pin.tile([128, 8, S], cdt, tag="xT")
            nc.sync.dma_start(xT[:], xT_d.ap().rearrange("(t p) s -> p t s", p=128))
            wq = pin.tile([128, 8, 256], cdt, tag="wq")
            nc.sync.dma_start(wq[:], wq_d.ap().rearrange("(t p) e -> p t e", p=128))
            wk = pin.tile([128, 8, 256], cdt, tag="wk")
            nc.sync.dma_start(wk[:], wk_d.ap().rearrange("(t p) e -> p t e", p=128))
            wv = pin.tile([128, 8, 256], cdt, tag="wv")
            nc.sync.dma_start(wv[:], wv_d.ap().rearrange("(t p) e -> p t e", p=128))
            wo = pin.tile([128, 2, D], cdt, tag="wo")
            nc.sync.dma_start(wo[:], wo_d.ap().rearrange("(t p) d -> p t d", p=128))
            mask0 = pin.tile([128, 2, 512], cdt, tag="mask0")
            nc.sync.dma_start(mask0[:], mask_d.ap())

            # persistent intermediates
            qT = pin.tile([128, NPAIR, S], cdt, tag="qT")
            kT = pin.tile([128, NPAIR, S], cdt, tag="kT")
            vsb = pin.tile([128, 16, HPC, 65], cdt, tag="vsb")
            outcat = pin.tile([128, NPAIR, S], cdt, tag="outcat")

            # prime the exp activation table load during the projection phase
            prime = work.tile([1, 1], F32, tag="prime")
            nc.scalar.activation(prime[:], mask0[0:1, 0, 0:1], EXP)

            def qk_proj(p):
                for w, dst in ((wq, qT), (wk, kT)):
                    for nch in range(4):
                        ps = psS.tile([128, 512], F32, tag="sps", name="psqk")
                        for kt in range(8):
                            mm(
                                ps[:],
                                w[:, kt, p * 128 : (p + 1) * 128],
                                xT[:, kt, nch * 512 : (nch + 1) * 512],
                                start=(kt == 0),
                                stop=(kt == 7),
                            )
                        nc.scalar.copy(dst[:, p, nch * 512 : (nch + 1) * 512], ps[:])

            def v_proj():
                # natural layout, all 4 heads, ones column appended per head
                for st in range(16):
                    ps = psS.tile([128, 256], F32, tag="sps", name="psv")
                    for kt in range(8):
                        mm(
                            ps[:],
                            xT[:, kt, st * 128 : (st + 1) * 128],
                            wv[:, kt, :],
                            start=(kt == 0),
                            stop=(kt == 7),
                        )
                    nc.vector.tensor_copy(
                        vsb[:, st, :, 0:64], ps[:].rearrange("p (h e) -> p h e", h=HPC)
                    )
                    nc.vector.memset(vsb[:, st, :, 64:65], 1.0)

            def attention(p):
                for qc in range(QC):
                    kmax = 4 * qc + 3
                    avs = [
                        psV.tile([65, 512], F32, tag="av", name=f"av{p}_{qc}_{h}")
                        for h in range(2)
                    ]
                    for kb in range(kmax + 1):
                        m = kb - 4 * qc
                        off = 128 * m if m > 0 else 0
                        n = 512 - off
                        # both heads' scores into one 2-bank psum tile
                        sps = psS.tile([128, 2, 512], F32, tag="sps", name="sps")
                        for h in range(2):
                            mm(
                                sps[:, h, 0:n],
                                kT[h * 64 : (h + 1) * 64, p, kb * 128 : (kb + 1) * 128],
                                qT[
                                    h * 64 : (h + 1) * 64,
                                    p,
                                    qc * 512 + off : (qc + 1) * 512,
                                ],
                            )
                        pt = work.tile([128, 2, 512], cdt, tag="pt")
                        nc.scalar.activation(
                            pt[:, :, 0:n], sps[:, :, 0:n], EXP, scale=0.125
                        )
                        if m >= 0:
                            nc.vector.tensor_mul(
                                pt[:, :, 0:n], pt[:, :, 0:n], mask0[:, :, 0:n]
                            )
                        for h in range(2):
                            mm(
                                avs[h][:, off:512],
                                vsb[:, kb, p * 2 + h, :],
                                pt[:, h, 0:n],
                                start=(kb == 0),
                                stop=(kb == kmax),
                            )
                    # normalize: outcat[h] = av[0:64] / av[64]
                    for h in range(2):
                        rec = work.tile([1, 512], F32, tag="rec")
                        nc.vector.reciprocal(rec[:], avs[h][64:65, :])
                        bc_sb = work.tile([64, 512], F32, tag="bcsb")
                        nc.gpsimd.partition_broadcast(bc_sb[:], rec[:])
                        nc.vector.tensor_mul(
                            outcat[h * 64 : (h + 1) * 64, p, qc * 512 : (qc + 1) * 512],
                            avs[h][0:64, :],
                            bc_sb[:],
                        )

            qk_proj(0)
            v_proj()
            attention(0)
            qk_proj(1)
            attention(1)

            # ---- output projection
            for st in range(16):
                for dc in range(2):
                    ps = psS.tile([128, 512], F32, tag="sps", name="pswo")
                    for p in range(NPAIR):
                        mm(
                            ps[:],
                            outcat[:, p, st * 128 : (st + 1) * 128],
                            wo[:, p, dc * 512 : (dc + 1) * 512],
                            start=(p == 0),
                            stop=(p == NPAIR - 1),
                        )
                    ysb = work.tile([128, 512], F32, tag="ysb")
                    nc.vector.tensor_copy(ysb[:], ps[:])
                    nc.sync.dma_start(
                        out_d.ap()[
                            st * 128 : (st + 1) * 128, dc * 512 : (dc + 1) * 512
                        ],
                        ysb[:],
                    )

            if iters > 1:
                loop_cm.__exit__(None, None, None)

    nc.compile()
    return nc


def _masks_np(dtype):
    """mask0[kl, h, f] = 1.0 where f >= kl else 0, duplicated per head pair."""
    kl = np.arange(128)[:, None]
    f = np.arange(512)[None, :]
    m = (f >= kl).astype(dtype)
    return np.stack([m, m], axis=1)


def kernel(x, Wq, Wk, Wv, Wo, bo):
    mode = MODE
    if mode not in _BUILD_CACHE:
        _BUILD_CACHE[mode] = _build(mode)
    nc = _BUILD_CACHE[mode]
    cdt = _np_cdt(mode)

    x = np.asarray(x, dtype=np.float32)
    Wq = np.asarray(Wq, dtype=np.float32)
    Wk = np.asarray(Wk, dtype=np.float32)
    Wv = np.asarray(Wv, dtype=np.float32)
    Wo = np.asarray(Wo, dtype=np.float32)
    bo = np.asarray(bo, dtype=np.float32)

    masks = _masks_np(cdt)
    in_maps = []
    for c in range(NCORES):
        b, j = divmod(c, 4)
        hs = slice(4 * j, 4 * j + 4)
        in_maps.append(
            {
                "xT": np.ascontiguousarray(x[b].T).astype(cdt),
                "wq": Wq[hs].transpose(1, 0, 2).reshape(D, HPC * HD).astype(cdt),
                "wk": Wk[hs].transpose(1, 0, 2).reshape(D, HPC * HD).astype(cdt),
                "wv": Wv[hs].transpose(1, 0, 2).reshape(D, HPC * HD).astype(cdt),
                "wo": np.ascontiguousarray(Wo[256 * j : 256 * (j + 1)]).astype(cdt),
                "mask0": masks,
            }
        )

    res = run_bass_kernel_spmd(nc, in_maps, core_ids=list(range(NCORES)))
    y = np.zeros((B, S, D), dtype=np.float32)
    for c in range(NCORES):
        y[c // 4] += res.results[c]["out"]
    y += bo
    return y
